# revision 1
# baseline (speedup 1.0000x reference)
"""Trainium2 Bass kernel for DeepRBF distance layer.

Computes distances[b, k] = || x[b] @ A[k] + bias[k] ||_2 for
x: (65536, 64), A: (64, 64, 64), bias: (64, 64) -> out (65536, 64).

Math (exact rewrite via the Gram matrices M_k = A_k A_k^T, symmetric):
  dist^2[b,k] = x_b M_k x_b^T + x_b . w_k + c_k
    w_k[d] = 2 sum_e A_k[d,e] bias[k,e],  c_k = ||bias_k||^2

The quadratic form is evaluated through "circulant plane" features:
  x M x^T = sum_{o=0..32} sum_d  Mt_o[d] * x_d * x_{(d+o)%64}
(pairs {d, d+o} enumerate all 2080 unordered index pairs).

Device strategy (per core, batch-sharded 8192 rows, everything
transposed: feature index on partitions, b on the free axis):
  - Host ships 9 "arrangement" tiles H_t = [rot_{F[t]} x^T ; rot_{G[t]} x^T]
    (128, 8192) fp16.  One DVE/GPSIMD tensor_tensor multiply per edge
    (i, j) of a difference-cover design produces TWO product planes at
    once (top: offset F[j]-F[i], bottom: G[j]-G[i]) at base partition 0
    (the only partition alignment the ISA allows: engines cannot read
    operands at different base partitions, so rotations are staged via
    DRAM instead).
  - 17 Z chunks (128, free) per superblock feed 17 accumulating
    matmuls (contraction 128 = 2 planes) per 512-col block into
    dist^2[k, b] PSUM.  Plane weights Mt (with rotated row labelings
    per edge) are host-precomputed; the 17-edge design covers each
    plane exactly once.
  - The self-edge chunk holds [x^2 (ACT Square) ; rot x (ACT copy)]
    covering the diagonal plane and the linear term w.
  - ACT applies sqrt (bias c_k per partition) -> (64, 512) fp16 out
    slabs; host transposes and upcasts.
The two 512-col halves of each superblock run interleaved-by-chunk on
the PE so each chunk's pair of matmuls fires as soon as the chunk is
ready (cuts the after-last-chunk tail from a full 17-MM group to 2).
Engine budget per core (cost model, ~73.8us span): PE ~63us (272
N=512 matmuls), DVE ~62us (13.75 edge-muls/superblock at 2x fp16),
GPSIMD ~50us, DMA ~56us (18MB arrangements + 1MB out), ACT ~29us.
"""

import sys

sys.path.insert(0, "/opt/trn_rl_repo")

import numpy as np

from concourse import bacc, bass_utils, mybir, tile

B, K, D = 65536, 64, 64
NCORES = 8
BC = B // NCORES            # 8192 cols per core
FREEW = 1024                # max superblock width (2 PSUM blocks of 512)
SB_WIDTHS = [1024] * 8
assert sum(SB_WIDTHS) == BC
NT = 9                      # arrangement tiles

# Difference-cover design: tile t holds [rot_{F[t]} x ; rot_{G[t]} x].
# Edge (i, j) covers plane F[j]-F[i] (top) and G[j]-G[i] (bottom), both
# folded mod 64 into 0..32.  17 edges cover all 33 planes; the self
# edge (3, 3) hosts [x^2 (rotated labeling) ; linear x].
F_LAB = [0, 3, 16, 19, 22, 24, 25, 50, 57]
G_LAB = [0, 3, 7, 27, 34, 35, 41, 45, 52]
# Edge order doubles as the matmul accumulation order and the engines'
# issue order: edges over early-loaded tiles (see LOAD_ORDER) come
# first so the pipeline fills fast; GPSIMD-assigned edges sit last.
EDGES = [(1, 2), (1, 3), (2, 3),
         (3, 3),
         (2, 4), (1, 5), (3, 5), (4, 5), (1, 6), (2, 6), (0, 6),
         (0, 7), (1, 8), (0, 8),
         (4, 7), (4, 8), (5, 8)]
SELF_EDGE = 3               # index into EDGES
LIN_TILE = 1                # arrangement tile whose top half feeds the
                            # linear slot (weights absorb its rotation)
LOAD_ORDER = [1, 2, 3, 4, 5, 6, 0, 7, 8]
NCH = len(EDGES)            # 17 chunks

# Engine assignment for the 16 product multiplies (edge index != SELF):
# "v" = DVE tensor_tensor, "g" = GPSIMD tensor_tensor.
MUL_ENGINE = {}
_gp_edges = {(4, 7), (4, 8), (5, 8)}
_split_edges = {(0, 8)}
for _q, _e in enumerate(EDGES):
    MUL_ENGINE[_q] = ("g" if _e in _gp_edges else
                      "s" if _e in _split_edges else "v")

F16 = mybir.dt.float16
F32 = mybir.dt.float32

_CACHE = {}


def _fold(d):
    d %= 64
    return min(d, 64 - d)


def _build_kernel():
    nc = bacc.Bacc("TRN2", target_bir_lowering=False, debug=False,
                   num_devices=NCORES)

    # All arrangement tiles stacked in one DRAM tensor: one load per
    # superblock feeds every tile slice.
    h_d = nc.dram_tensor("h", [128, NT * BC], F16, kind="ExternalInput").ap()
    w_d = nc.dram_tensor("w", [128, NCH * K], F16, kind="ExternalInput").ap()
    c_d = nc.dram_tensor("c", [K, 1], F32, kind="ExternalInput").ap()
    out_d = nc.dram_tensor("out", [K, BC], F16, kind="ExternalOutput").ap()

    SQUARE = mybir.ActivationFunctionType.Square
    SQRT = mybir.ActivationFunctionType.Sqrt
    MULT = mybir.AluOpType.mult

    with tile.TileContext(nc) as tc:
        with tc.tile_pool(name="const", bufs=1) as cpool, \
             tc.tile_pool(name="arr", bufs=3) as apool, \
             tc.tile_pool(name="z", bufs=3) as zpool, \
             tc.tile_pool(name="dps", bufs=4, space="PSUM") as dpool, \
             tc.tile_pool(name="ost", bufs=6) as opool:

            w_sb = cpool.tile([128, NCH * K], F16)
            c_sb = cpool.tile([K, 1], F32)

            h3 = h_d.rearrange("p (t c) -> p t c", t=NT)
            c0 = 0
            for sb, wid in enumerate(SB_WIDTHS):
                arr = [None] * NT
                for n, t in enumerate(LOAD_ORDER):
                    a = apool.tile([128, wid], F16, tag=f"a{t}",
                                   padded_shape=[128, FREEW], name=f"a{t}")
                    nc.sync.dma_start(a[:], h3[:, t, c0:c0 + wid])
                    arr[t] = a[:]
                    if sb == 0 and n == 2:
                        # stationaries slot in after the tiles the first
                        # muls need, but before the PE's first group
                        nc.sync.dma_start(w_sb[:], w_d[:])
                        nc.sync.dma_start(c_sb[:], c_d[:])

                # In the last superblock Pool would finish well after DVE
                # (pipeline drain); give its edges to the otherwise-idle DVE.
                last_sb = sb == len(SB_WIDTHS) - 1
                zs = []
                for q, (i, j) in enumerate(EDGES):
                    z = zpool.tile([128, wid], F16, tag=f"z{q}",
                                   padded_shape=[128, FREEW])
                    eng = MUL_ENGINE[q]
                    if last_sb and eng == "g":
                        eng = "v"
                    if q == SELF_EDGE:
                        # top: x^2 on ACT (rotated labeling of tile i);
                        # bottom: linear slot via ACT copy of an early-
                        # loaded tile (weights absorb its rotation)
                        nc.scalar.activation(z[0:64, :], arr[i][0:64, :],
                                             SQUARE)
                        nc.scalar.copy(out=z[64:128, :],
                                       in_=arr[LIN_TILE][0:64, :])
                    elif eng == "g":
                        nc.gpsimd.tensor_tensor(
                            out=z[:], in0=arr[i], in1=arr[j], op=MULT)
                    elif eng == "s" and wid > 512:
                        # column-split across DVE and GPSIMD for balance
                        sp = wid - 256
                        nc.vector.tensor_tensor(
                            out=z[:, 0:sp], in0=arr[i][:, 0:sp],
                            in1=arr[j][:, 0:sp], op=MULT)
                        nc.gpsimd.tensor_tensor(
                            out=z[:, sp:wid], in0=arr[i][:, sp:wid],
                            in1=arr[j][:, sp:wid], op=MULT)
                    else:
                        nc.vector.tensor_tensor(
                            out=z[:], in0=arr[i], in1=arr[j], op=MULT)
                    zs.append(z)

                # Interleave the halves' accumulation groups by chunk: a
                # chunk's two MMs run back-to-back as soon as it is ready,
                # so the tail after the last chunk is 2 MMs, not a full
                # 17-MM group replay.
                nhalf = wid // 512
                d2s = [dpool.tile([K, 512], F32, tag=f"d2{h}", name=f"d2{h}")
                       for h in range(nhalf)]
                for q in range(NCH):
                    for half in range(nhalf):
                        bsl = slice(512 * half, 512 * (half + 1))
                        nc.tensor.matmul(d2s[half][:],
                                         lhsT=w_sb[:, K * q:K * (q + 1)],
                                         rhs=zs[q][:, bsl],
                                         start=(q == 0), stop=(q == NCH - 1),
                                         skip_group_check=True)
                for half in range(nhalf):
                    o = opool.tile([K, 512], F16)
                    nc.scalar.activation(o[:], d2s[half][:], SQRT,
                                         bias=c_sb[:])
                    # ACT's own HWDGE queue: keeps the out-DMA's wait from
                    # head-of-line-blocking the SP queue that feeds loads.
                    nc.scalar.dma_start(
                        out_d[:, c0 + 512 * half:c0 + 512 * (half + 1)],
                        o[:])
                c0 += wid

    nc.compile()
    return nc


def _prepare_inputs(x, A, b):
    """Host-side prep: Gram matrices, plane weights, arrangement tiles."""
    x = np.asarray(x, dtype=np.float32)
    A = np.asarray(A, dtype=np.float32)
    b = np.asarray(b, dtype=np.float32)

    xt = np.ascontiguousarray(x.T).astype(np.float16)           # (D, B)
    M = np.einsum("kde,kfe->kdf", A, A)                         # (K, D, D)
    w = 2.0 * np.einsum("kde,ke->kd", A, b)                     # (K, D)
    c = (b * b).sum(axis=1).astype(np.float32).reshape(K, 1)

    # Plane multiplicity for weight splitting across duplicate slots.
    n_cov = np.zeros(33, dtype=np.int64)
    for q, (i, j) in enumerate(EDGES):
        if q == SELF_EDGE:
            n_cov[0] += 1                       # top slot only
        else:
            n_cov[_fold(F_LAB[j] - F_LAB[i])] += 1
            n_cov[_fold(G_LAB[j] - G_LAB[i])] += 1

    def slot_weights(mi, mj):
        """(64, K) weights for one slot: rows p, pair ((p+mi)%64,(p+mj)%64)."""
        plane = _fold(mj - mi)
        p = np.arange(64)
        d_idx = (p + mi) % 64
        e_idx = (p + mj) % 64
        wt = M[:, d_idx, e_idx].T                               # (64, K)
        if plane == 0:
            gamma = 1.0 / n_cov[0]
        elif plane == 32:
            gamma = 1.0 / n_cov[32]            # each pair appears twice/slot
        else:
            gamma = 2.0 / n_cov[plane]
        return gamma * wt

    wst = np.zeros((128, NCH * K), dtype=np.float32)
    for q, (i, j) in enumerate(EDGES):
        if q == SELF_EDGE:
            wst[0:64, K * q:K * (q + 1)] = slot_weights(F_LAB[i], F_LAB[i])
            # linear term, relabeled for the rot_{F[LIN_TILE]} copy source
            wst[64:128, K * q:K * (q + 1)] = np.roll(
                w.T, -F_LAB[LIN_TILE], axis=0)                  # (64, K)
        else:
            wst[0:64, K * q:K * (q + 1)] = slot_weights(F_LAB[i], F_LAB[j])
            wst[64:128, K * q:K * (q + 1)] = slot_weights(G_LAB[i], G_LAB[j])
    wst = wst.astype(np.float16)

    in_maps = []
    for s in range(NCORES):
        xc = np.ascontiguousarray(xt[:, s * BC:(s + 1) * BC])
        h = np.empty((128, NT * BC), dtype=np.float16)
        for t in range(NT):
            h[0:64, t * BC:(t + 1) * BC] = np.roll(xc, -F_LAB[t], axis=0)
            h[64:128, t * BC:(t + 1) * BC] = np.roll(xc, -G_LAB[t], axis=0)
        in_maps.append({"h": h, "w": wst, "c": c})
    return in_maps


def _run(in_maps, trace=False, **kw):
    if "nc" not in _CACHE:
        _CACHE["nc"] = _build_kernel()
    nc = _CACHE["nc"]
    return bass_utils.run_bass_kernel_spmd(
        nc, in_maps, core_ids=list(range(NCORES)), trace=trace, **kw)


def _postprocess(res):
    out = np.concatenate(
        [res.results[s]["out"].astype(np.float32).T for s in range(NCORES)],
        axis=0)
    return np.ascontiguousarray(out)


def kernel(x, A, b):
    in_maps = _prepare_inputs(x, A, b)
    res = _run(in_maps)
    return _postprocess(res)



# revision 2
# speedup vs baseline: 1.2150x; 1.2150x over previous
"""Trainium2 Bass kernel for DeepRBF distance layer.

Computes distances[b, k] = || x[b] @ A[k] + bias[k] ||_2 for
x: (65536, 64), A: (64, 64, 64), bias: (64, 64) -> out (65536, 64).

Math (exact rewrite via the Gram matrices M_k = A_k A_k^T, symmetric):
  dist^2[b,k] = x_b M_k x_b^T + x_b . w_k + c_k
    w_k[d] = 2 sum_e A_k[d,e] bias[k,e],  c_k = ||bias_k||^2

The quadratic form is evaluated through "circulant plane" features:
  x M x^T = sum_{o=0..32} sum_d  Mt_o[d] * x_d * x_{(d+o)%64}
(pairs {d, d+o} enumerate all 2080 unordered index pairs).

Device strategy (per core, batch-sharded 8192 rows, everything
transposed: feature index on partitions, b on the free axis):
  - Host ships 9 "arrangement" tiles H_t = [rot_{F[t]} x^T ; rot_{G[t]} x^T]
    (128, 8192) fp16.  One DVE/GPSIMD tensor_tensor multiply per edge
    (i, j) of a difference-cover design produces TWO product planes at
    once (top: offset F[j]-F[i], bottom: G[j]-G[i]) at base partition 0
    (the only partition alignment the ISA allows: engines cannot read
    operands at different base partitions, so rotations are staged via
    DRAM instead).
  - 17 Z chunks (128, free) per superblock feed 17 accumulating
    matmuls (contraction 128 = 2 planes) per 512-col block into
    dist^2[k, b] PSUM.  Plane weights Mt (with rotated row labelings
    per edge) are host-precomputed; the 17-edge design covers each
    plane exactly once.
  - The self-edge chunk holds [x^2 (ACT Square) ; rot x (ACT copy)]
    covering the diagonal plane and the linear term w.
  - ACT applies sqrt (bias c_k per partition) -> (64, 512) fp16 out
    slabs; host transposes and upcasts.
The two 512-col halves of each superblock run interleaved-by-chunk on
the PE so each chunk's pair of matmuls fires as soon as the chunk is
ready (cuts the after-last-chunk tail from a full 17-MM group to 2).
Engine budget per core (cost model, ~73.8us span): PE ~63us (272
N=512 matmuls), DVE ~62us (13.75 edge-muls/superblock at 2x fp16),
GPSIMD ~50us, DMA ~56us (18MB arrangements + 1MB out), ACT ~29us.
"""

import sys

sys.path.insert(0, "/opt/trn_rl_repo")

import ml_dtypes
import numpy as np

from concourse import bacc, bass_utils, mybir, tile

B, K, D = 65536, 64, 64
NCORES = 8
BC = B // NCORES            # 8192 cols per core
FREEW = 1024                # max superblock width (2 PSUM blocks of 512)
SB_WIDTHS = [1024] * 8
assert sum(SB_WIDTHS) == BC
NT = 9                      # arrangement tiles

# Difference-cover design: tile t holds [rot_{F[t]} x ; rot_{G[t]} x].
# Edge (i, j) covers plane F[j]-F[i] (top) and G[j]-G[i] (bottom), both
# folded mod 64 into 0..32.  17 edges cover all 33 planes; the self
# edge (3, 3) hosts [x^2 (rotated labeling) ; linear x].
F_LAB = [0, 3, 16, 19, 22, 24, 25, 50, 57]
G_LAB = [0, 3, 7, 27, 34, 35, 41, 45, 52]
# Edge order doubles as the matmul accumulation order and the engines'
# issue order: edges over early-loaded tiles (see LOAD_ORDER) come
# first so the pipeline fills fast; GPSIMD-assigned edges sit last.
EDGES = [(1, 2), (1, 3), (2, 3),
         (3, 3),
         (2, 4), (1, 5), (3, 5), (4, 5), (1, 6), (2, 6), (0, 6),
         (0, 7), (1, 8), (0, 8),
         (4, 7), (4, 8), (5, 8)]
SELF_EDGE = 3               # index into EDGES
LIN_TILE = 1                # arrangement tile whose top half feeds the
                            # linear slot (weights absorb its rotation)
LOAD_ORDER = [1, 2, 3, 4, 5, 6, 0, 7, 8]
NCH = len(EDGES)            # 17 chunks

# Engine assignment for the 16 product multiplies (edge index != SELF):
# "v" = DVE tensor_tensor, "g" = GPSIMD tensor_tensor.
MUL_ENGINE = {}
_gp_edges = {(4, 7), (4, 8), (5, 8)}
_split_edges = {(0, 8)}
for _q, _e in enumerate(EDGES):
    MUL_ENGINE[_q] = ("g" if _e in _gp_edges else
                      "s" if _e in _split_edges else "v")

F16 = mybir.dt.float16
BF16 = mybir.dt.bfloat16
F32 = mybir.dt.float32

_CACHE = {}


def _fold(d):
    d %= 64
    return min(d, 64 - d)


def _build_kernel():
    nc = bacc.Bacc("TRN2", target_bir_lowering=False, debug=False,
                   num_devices=NCORES)

    # All arrangement tiles stacked in one DRAM tensor: one load per
    # superblock feeds every tile slice.
    h_d = nc.dram_tensor("h", [128, NT * BC], BF16, kind="ExternalInput").ap()
    w_d = nc.dram_tensor("w", [128, NCH * K], BF16, kind="ExternalInput").ap()
    c_d = nc.dram_tensor("c", [K, 1], F32, kind="ExternalInput").ap()
    out_d = nc.dram_tensor("out", [K, BC], F16, kind="ExternalOutput").ap()

    SQUARE = mybir.ActivationFunctionType.Square
    SQRT = mybir.ActivationFunctionType.Sqrt
    MULT = mybir.AluOpType.mult

    with tile.TileContext(nc) as tc:
        with tc.tile_pool(name="const", bufs=1) as cpool, \
             tc.tile_pool(name="arr", bufs=3) as apool, \
             tc.tile_pool(name="z", bufs=3) as zpool, \
             tc.tile_pool(name="dps", bufs=4, space="PSUM") as dpool, \
             tc.tile_pool(name="ost", bufs=6) as opool:

            w_sb = cpool.tile([128, NCH * K], BF16)
            c_sb = cpool.tile([K, 1], F32)

            h3 = h_d.rearrange("p (t c) -> p t c", t=NT)
            c0 = 0
            for sb, wid in enumerate(SB_WIDTHS):
                arr = [None] * NT
                for n, t in enumerate(LOAD_ORDER):
                    a = apool.tile([128, wid], BF16, tag=f"a{t}",
                                   padded_shape=[128, FREEW], name=f"a{t}")
                    nc.sync.dma_start(a[:], h3[:, t, c0:c0 + wid])
                    arr[t] = a[:]
                    if sb == 0 and n == 2:
                        # stationaries slot in after the tiles the first
                        # muls need, but before the PE's first group
                        nc.sync.dma_start(w_sb[:], w_d[:])
                        nc.sync.dma_start(c_sb[:], c_d[:])

                # In the last superblock Pool would finish well after DVE
                # (pipeline drain); give its edges to the otherwise-idle DVE.
                last_sb = sb == len(SB_WIDTHS) - 1
                zs = []
                for q, (i, j) in enumerate(EDGES):
                    z = zpool.tile([128, wid], BF16, tag=f"z{q}",
                                   padded_shape=[128, FREEW])
                    eng = MUL_ENGINE[q]
                    if last_sb and eng == "g":
                        eng = "v"
                    if q == SELF_EDGE:
                        # top: x^2 on ACT (rotated labeling of tile i);
                        # bottom: linear slot via ACT copy of an early-
                        # loaded tile (weights absorb its rotation)
                        nc.scalar.activation(z[0:64, :], arr[i][0:64, :],
                                             SQUARE)
                        nc.scalar.copy(out=z[64:128, :],
                                       in_=arr[LIN_TILE][0:64, :])
                    elif eng == "g":
                        nc.gpsimd.tensor_tensor(
                            out=z[:], in0=arr[i], in1=arr[j], op=MULT)
                    elif eng == "s" and wid > 512:
                        # column-split across DVE and GPSIMD for balance
                        sp = wid - 256
                        nc.vector.tensor_tensor(
                            out=z[:, 0:sp], in0=arr[i][:, 0:sp],
                            in1=arr[j][:, 0:sp], op=MULT)
                        nc.gpsimd.tensor_tensor(
                            out=z[:, sp:wid], in0=arr[i][:, sp:wid],
                            in1=arr[j][:, sp:wid], op=MULT)
                    else:
                        nc.vector.tensor_tensor(
                            out=z[:], in0=arr[i], in1=arr[j], op=MULT)
                    zs.append(z)

                # Interleave the halves' accumulation groups by chunk: a
                # chunk's two MMs run back-to-back as soon as it is ready,
                # so the tail after the last chunk is 2 MMs, not a full
                # 17-MM group replay.
                nhalf = wid // 512
                d2s = [dpool.tile([K, 512], F32, tag=f"d2{h}", name=f"d2{h}")
                       for h in range(nhalf)]
                for q in range(NCH):
                    for half in range(nhalf):
                        bsl = slice(512 * half, 512 * (half + 1))
                        nc.tensor.matmul(d2s[half][:],
                                         lhsT=w_sb[:, K * q:K * (q + 1)],
                                         rhs=zs[q][:, bsl],
                                         start=(q == 0), stop=(q == NCH - 1),
                                         skip_group_check=True)
                for half in range(nhalf):
                    o = opool.tile([K, 512], F16)
                    nc.scalar.activation(o[:], d2s[half][:], SQRT,
                                         bias=c_sb[:])
                    # ACT's own HWDGE queue: keeps the out-DMA's wait from
                    # head-of-line-blocking the SP queue that feeds loads.
                    nc.scalar.dma_start(
                        out_d[:, c0 + 512 * half:c0 + 512 * (half + 1)],
                        o[:])
                c0 += wid

    nc.compile()
    return nc


def _prepare_inputs(x, A, b):
    """Host-side prep: Gram matrices, plane weights, arrangement tiles."""
    x = np.asarray(x, dtype=np.float32)
    A = np.asarray(A, dtype=np.float32)
    b = np.asarray(b, dtype=np.float32)

    xt = np.ascontiguousarray(x.T).astype(ml_dtypes.bfloat16)   # (D, B)
    M = np.einsum("kde,kfe->kdf", A, A)                         # (K, D, D)
    w = 2.0 * np.einsum("kde,ke->kd", A, b)                     # (K, D)
    c = (b * b).sum(axis=1).astype(np.float32).reshape(K, 1)

    # Plane multiplicity for weight splitting across duplicate slots.
    n_cov = np.zeros(33, dtype=np.int64)
    for q, (i, j) in enumerate(EDGES):
        if q == SELF_EDGE:
            n_cov[0] += 1                       # top slot only
        else:
            n_cov[_fold(F_LAB[j] - F_LAB[i])] += 1
            n_cov[_fold(G_LAB[j] - G_LAB[i])] += 1

    def slot_weights(mi, mj):
        """(64, K) weights for one slot: rows p, pair ((p+mi)%64,(p+mj)%64)."""
        plane = _fold(mj - mi)
        p = np.arange(64)
        d_idx = (p + mi) % 64
        e_idx = (p + mj) % 64
        wt = M[:, d_idx, e_idx].T                               # (64, K)
        if plane == 0:
            gamma = 1.0 / n_cov[0]
        elif plane == 32:
            gamma = 1.0 / n_cov[32]            # each pair appears twice/slot
        else:
            gamma = 2.0 / n_cov[plane]
        return gamma * wt

    wst = np.zeros((128, NCH * K), dtype=np.float32)
    for q, (i, j) in enumerate(EDGES):
        if q == SELF_EDGE:
            wst[0:64, K * q:K * (q + 1)] = slot_weights(F_LAB[i], F_LAB[i])
            # linear term, relabeled for the rot_{F[LIN_TILE]} copy source
            wst[64:128, K * q:K * (q + 1)] = np.roll(
                w.T, -F_LAB[LIN_TILE], axis=0)                  # (64, K)
        else:
            wst[0:64, K * q:K * (q + 1)] = slot_weights(F_LAB[i], F_LAB[j])
            wst[64:128, K * q:K * (q + 1)] = slot_weights(G_LAB[i], G_LAB[j])
    wst = wst.astype(ml_dtypes.bfloat16)

    in_maps = []
    for s in range(NCORES):
        xc = np.ascontiguousarray(xt[:, s * BC:(s + 1) * BC])
        h = np.empty((128, NT * BC), dtype=ml_dtypes.bfloat16)
        for t in range(NT):
            h[0:64, t * BC:(t + 1) * BC] = np.roll(xc, -F_LAB[t], axis=0)
            h[64:128, t * BC:(t + 1) * BC] = np.roll(xc, -G_LAB[t], axis=0)
        in_maps.append({"h": h, "w": wst, "c": c})
    return in_maps


def _run(in_maps, trace=False, **kw):
    if "nc" not in _CACHE:
        _CACHE["nc"] = _build_kernel()
    nc = _CACHE["nc"]
    return bass_utils.run_bass_kernel_spmd(
        nc, in_maps, core_ids=list(range(NCORES)), trace=trace, **kw)


def _postprocess(res):
    out = np.concatenate(
        [res.results[s]["out"].astype(np.float32).T for s in range(NCORES)],
        axis=0)
    return np.ascontiguousarray(out)


def kernel(x, A, b):
    in_maps = _prepare_inputs(x, A, b)
    res = _run(in_maps)
    return _postprocess(res)

